# revision 1
# baseline (speedup 1.0000x reference)
"""BlockAttentionResidual Trainium2 kernel (bf16 edition).

Math (per token t, feature dim D=1024, over N+1=9 blocks):
    ssq[n,t]  = sum_d v[n,t,d]^2
    rq[n,t]   = (ssq/D + eps)^(-1/2)        (computed as exp(-0.5*ln(ssq/D+eps)))
    logit     = (sum_d w2[d]*v[n,t,d]) * rq      where w2 = proj_w*norm_w
    w[n,t]    = softmax over n of logit
    h[t,d]    = sum_n w[n,t] * v[n,t,d]

Sharding: B*T = 8192 tokens split evenly across 8 cores (1024 tokens/core).

The host casts all bulk data to bf16 (tolerance is 2e-2; bf16 keeps the
result ~6e-3), which halves HBM traffic: input 18.9MB + output 2.1MB per
core, so the DMA stream (~6us/oct) stays well under the compute rate and
the kernel is compute-bound on DVE+ACT (~11us/oct each, balanced).

NOTE: the accumulating DVE ops (scalar_tensor_tensor / tensor_scalar
with accum_out) run at 1x on HW regardless of dtype — the 2x/4x perf
modes have no uops for the accumulating variants (the rust cost model
claims otherwise; hardware says no). So each stat is ONE 1x stt pass.

Host-side prep: per core the 9 blocks are pre-interleaved into
vstack[quad, p, (g,d)] (bf16) where partition p = 14*n + t' stacks the 9
blocks of 14 tokens (126 rows) and the free dim holds 8 such token-groups
(one "oct" = 112 tokens). Each oct's input is a contiguous [126, 8192]
bf16 DMA with 16KB-per-partition descriptors.

Per-oct on-chip pipeline:
  - dot:  DVE scalar_tensor_tensor (v*1)*w2b with accum_out   (1 pass)
  - ssq:  ACT activation(Square, accum_out) for ACT_SSQ groups,
          DVE stt (v*1)*v with accum_out for the rest (engine balance)
  - softmax over n: TensorE matmuls against a 0/1 mask M[p,t'] = (p%14==t')
      Z = M^T @ exp(logits), and M @ (1/Z) broadcasts 1/Z back to rows.
  - h:    TensorE bf16 matmul  h[t',d] = sum_p lhsT[p,t'] * v[p,d]
      with lhsT = M * w_col, 4 groups packed into one [128,1024] PSUM page
      at partition offsets 0/32/64/96 (PE column-group tiling).
  - PSUM -> SBUF copy (fp32 -> bf16 cast, on ACT), then bf16 DMA out
    (per 14-row group: SWDGE/Q7 mid-run, Sync HWDGE for the last octs);
    host upcasts.

Scheduling (the big wins over the naive loop):
  - software pipeline with the previous oct's softmax chain + copies
    emitted INSIDE the next oct's stat-pass stream (hooks after groups 1
    and 5) so every engine's static order always has ready work;
  - input DMA triggers lead by 3 octs so they never queue behind
    compute-dependent triggers on the Sync queue;
  - generous SBUF tile rings (vq x6, h_sb x6) so the paced output DMAs
    never starve the copy stage.
"""

import os
import sys
import numpy as np

for _p in ("/opt/trn_rl_repo", "/root/.axon_site/_ro/trn_rl_repo"):
    if os.path.isdir(_p) and _p not in sys.path:
        sys.path.append(_p)

N_CORES = 8
N, B, T, D = 8, 4, 2048, 1024
EPS = 1e-6
TOK = (B * T) // N_CORES          # 1024 tokens per core
NB = N + 1                        # 9 stacked blocks
GROUP = 14                        # tokens per group (14*9 = 126 <= 128)
ROWS = GROUP * NB                 # 126 used partitions
QG = 8                            # groups per oct (two PSUM pages)
QTOK = GROUP * QG                 # 112 tokens per oct
NQUAD = (TOK + QTOK - 1) // QTOK  # 10 (last oct ragged: 16 real tokens)

# groups whose ssq runs on ACT (rest on DVE): engine balance knob
ACT_SSQ = int(os.environ.get("BLOCKATTN_ACT_SSQ", "7"))
# columns of each PSUM page copied by ACT (rest by DVE)
ACT_COPY_COLS = int(os.environ.get("BLOCKATTN_ACT_COPY", "1024"))
ACT_SET = "natural_log_exp_and_others"

_CACHE = {}


def _groups(q):
    """[(g, t0, tg)] active groups of quad q (t0 = core-local token base)."""
    out = []
    for g in range(QG):
        t0 = q * QTOK + g * GROUP
        tg = min(GROUP, TOK - t0)
        if tg > 0:
            out.append((g, t0, tg))
    return out


def _patch_act_tables():
    """Make every activation func this kernel uses resolve to one table set
    (ACT_SET), so bacc emits a single ACT_TABLE_LOAD instead of thrashing
    between sets on every Ln/Exp/Square transition."""
    import concourse.bacc as bacc_mod
    import concourse.hw_specs as hw_specs
    from concourse import mybir

    if getattr(bacc_mod, "_blockattn_act_patch", False):
        return
    AF = mybir.ActivationFunctionType
    mine = {AF.Square, AF.Exp, AF.Ln, AF.Copy, AF.Identity}
    orig = hw_specs.get_activation_tables

    def patched(arch):
        t = dict(orig(arch))
        assert ACT_SET in t and mine <= t[ACT_SET], (ACT_SET, t.get(ACT_SET))
        return {
            name: (funcs if name == ACT_SET else funcs - mine)
            for name, funcs in t.items()
        }

    bacc_mod.get_activation_tables = patched
    bacc_mod._blockattn_act_patch = True


def build_nc():
    import concourse.bacc as bacc
    import concourse.tile as tile
    from concourse import mybir

    _patch_act_tables()

    f32 = mybir.dt.float32
    bf16 = mybir.dt.bfloat16
    AF = mybir.ActivationFunctionType
    OP = mybir.AluOpType

    nc = bacc.Bacc("TRN2", target_bir_lowering=False, debug=False)

    vst_d = nc.dram_tensor("vstack", [NQUAD, ROWS, QG * D], bf16,
                           kind="ExternalInput")
    w2b_d = nc.dram_tensor("w2b", [ROWS, D], bf16, kind="ExternalInput")
    oh_d = nc.dram_tensor("onehot", [ROWS, GROUP], f32, kind="ExternalInput")
    ohT_d = nc.dram_tensor("onehotT", [GROUP, ROWS], f32, kind="ExternalInput")
    oh8_d = nc.dram_tensor("onehot8", [ROWS, QG * GROUP], bf16,
                           kind="ExternalInput")
    h_d = nc.dram_tensor("h", [TOK, D], bf16, kind="ExternalOutput")

    vst = vst_d.ap()
    hout = h_d.ap()

    with tile.TileContext(nc) as tc:
        import contextlib
        ctx = contextlib.ExitStack()
        with ctx:
            consts = ctx.enter_context(tc.tile_pool(name="consts", bufs=1))
            vq_pool = ctx.enter_context(tc.tile_pool(name="vq", bufs=6))
            scr_pool = ctx.enter_context(tc.tile_pool(name="scr", bufs=4))
            stats_pool = ctx.enter_context(tc.tile_pool(name="stats", bufs=6))
            small_pool = ctx.enter_context(tc.tile_pool(name="small", bufs=4))
            hsb_pool = ctx.enter_context(tc.tile_pool(name="hsb", bufs=6))
            hpage_pool = ctx.enter_context(
                tc.tile_pool(name="hpage", bufs=3, space="PSUM"))
            zp_pool = ctx.enter_context(
                tc.tile_pool(name="zp", bufs=1, space="PSUM"))
            rzb_pool = ctx.enter_context(
                tc.tile_pool(name="rzb", bufs=1, space="PSUM"))

            w2b = consts.tile([ROWS, D], bf16)
            nc.sync.dma_start(w2b[:], w2b_d.ap()[:])
            oh = consts.tile([ROWS, GROUP], f32)
            nc.sync.dma_start(oh[:], oh_d.ap()[:])
            ohT = consts.tile([GROUP, ROWS], f32)
            nc.sync.dma_start(ohT[:], ohT_d.ap()[:])
            oh8 = consts.tile([ROWS, QG * GROUP], bf16)
            nc.sync.dma_start(oh8[:], oh8_d.ap()[:])
            zero_col = consts.tile([ROWS, 1], f32)
            nc.vector.memset(zero_col[:], 0.0)
            eps_col = consts.tile([ROWS, 1], f32)
            nc.vector.memset(eps_col[:], EPS)

            def emit_load(q):
                """Allocate tiles + input DMA for quad q (tail oct: only
                the active groups' columns; first octs chunked so stats can
                start before the whole slab lands)."""
                groups = _groups(q)
                vq = vq_pool.tile([ROWS, QG * D], bf16)
                stats = stats_pool.tile([ROWS, 2 * QG], f32)
                n_chunks = 4 if q == 0 else (2 if q < NQUAD - 1 else 1)
                if n_chunks > 1:
                    cw = len(groups) * D // n_chunks
                    for ci in range(n_chunks):
                        nc.sync.dma_start(vq[:, ci * cw:(ci + 1) * cw],
                                          vst[q][:, ci * cw:(ci + 1) * cw])
                else:
                    used = len(groups) * D
                    nc.sync.dma_start(vq[:, 0:used], vst[q][:, 0:used])
                return vq, stats

            def emit_passes(q, vq, stats, part1=None, part2=None):
                """The 16 heavy stat passes for quad q.

                part1/part2 are emission hooks for the previous quad's
                small-op chain and copy/out stages: interleaving them into
                this quad's pass stream puts them early in each engine's
                static order, so outputs start while compute still runs.
                """
                groups = _groups(q)
                # tail quad: ssq all on ACT (it idles at the end while DVE
                # is the critical engine); two mid quads shed one square to
                # DVE for global balance
                if q == NQUAD - 1:
                    act_ssq = QG
                elif q in (3, 6):
                    act_ssq = ACT_SSQ - 1
                else:
                    act_ssq = ACT_SSQ
                for i, (g, t0, tg) in enumerate(groups):
                    gc = g * D
                    # dot: single 1x stt pass with accumulate (the DVE's
                    # 2x/4x modes have no uop for the accumulating variants)
                    u_scr = scr_pool.tile([ROWS, D], bf16, tag="u_scr")
                    nc.vector.scalar_tensor_tensor(
                        out=u_scr[0:ROWS, :], in0=vq[0:ROWS, gc:gc + D],
                        scalar=1.0, in1=w2b[0:ROWS, :],
                        op0=OP.mult, op1=OP.mult,
                        accum_out=stats[:, QG + g:QG + g + 1])
                    # ssq: ACT full-service for act_ssq groups, else DVE
                    if g < act_ssq:
                        sqa = scr_pool.tile([ROWS, D], bf16, tag="sqa")
                        nc.scalar.activation(
                            sqa[0:ROWS, :], vq[0:ROWS, gc:gc + D], AF.Square,
                            bias=zero_col[:], accum_out=stats[:, g:g + 1])
                    else:
                        sq = scr_pool.tile([ROWS, D], bf16, tag="sq")
                        nc.vector.scalar_tensor_tensor(
                            out=sq[0:ROWS, :], in0=vq[0:ROWS, gc:gc + D],
                            scalar=1.0, in1=vq[0:ROWS, gc:gc + D],
                            op0=OP.mult, op1=OP.mult,
                            accum_out=stats[:, g:g + 1])
                    if i == 1 and part1 is not None:
                        part1()
                        part1 = None
                    if i == 5 and part2 is not None:
                        part2()
                        part2 = None
                if part1 is not None:
                    part1()
                if part2 is not None:
                    part2()

            def emit_chain(q, vq, stats):
                """Softmax smalls + PE weighted sum (part 1 of the chain)."""
                groups = _groups(q)
                lnq = small_pool.tile([ROWS, QG], f32, tag="lnq")
                nc.scalar.activation(lnq[:], stats[:, 0:QG], AF.Ln,
                                     bias=eps_col[:], scale=1.0 / D)
                rq = small_pool.tile([ROWS, QG], f32, tag="rq")
                nc.scalar.activation(rq[:], lnq[:], AF.Exp,
                                     bias=zero_col[:], scale=-0.5)
                lg = small_pool.tile([ROWS, QG], f32, tag="lg")
                nc.vector.tensor_mul(lg[:], stats[:, QG:2 * QG], rq[:])
                e_sb = small_pool.tile([ROWS, QG], f32, tag="e_sb")
                nc.scalar.activation(e_sb[:], lg[:], AF.Exp, bias=zero_col[:])

                zp = zp_pool.tile([GROUP, QG], f32)
                nc.tensor.matmul(zp[:], lhsT=oh[:], rhs=e_sb[:],
                                 start=True, stop=True)
                rz = small_pool.tile([GROUP, QG], f32, tag="rz")
                nc.vector.reciprocal(rz[:], zp[:])
                rzb = rzb_pool.tile([ROWS, QG], f32)
                nc.tensor.matmul(rzb[:], lhsT=ohT[:], rhs=rz[:],
                                 start=True, stop=True)
                wcol = small_pool.tile([ROWS, QG], bf16, tag="wcol")
                nc.vector.tensor_mul(wcol[:], e_sb[:], rzb[:])

                # ---- weighted sum via PE (bf16), 4 groups per PSUM page ----
                lhsTs = small_pool.tile([ROWS, QG * GROUP], bf16, tag="lhsTs")
                active_pages = sorted({g // 4 for g, _, _ in groups})
                hpages = {pg: hpage_pool.tile([128, D], f32, tag="hpage",
                                              name="hpage")
                          for pg in active_pages}
                nc.vector.tensor_tensor(
                    out=lhsTs[:, :].rearrange("p (g j) -> p g j", g=QG),
                    in0=oh8[:, :].rearrange("p (g j) -> p g j", g=QG),
                    in1=wcol[:, :].unsqueeze(2).to_broadcast(
                        [ROWS, QG, GROUP]),
                    op=OP.mult)
                for g, t0, tg in groups:
                    gc = g * D
                    lw = lhsTs[:, g * GROUP:(g + 1) * GROUP]
                    pg = g // 4
                    col = 32 * (g % 4)
                    for hh in range(2):
                        nc.tensor.matmul(
                            hpages[pg][col:col + GROUP,
                                       512 * hh:512 * hh + 512],
                            lhsT=lw,
                            rhs=vq[0:ROWS, gc + 512 * hh:gc + 512 * hh + 512],
                            start=True, stop=True,
                            tile_position=(0, col))

                return hpages, active_pages

            def emit_copies(q, vq, hpages, active_pages):
                """PSUM -> SBUF (f32 -> bf16, split ACT/DVE) -> HBM."""
                groups = _groups(q)
                # last quads: copies fully on ACT (it idles at the end while
                # DVE finishes the tail stats), and output triggers split
                # between the Sync HWDGE queue and the Q7 so the ~0.7-0.85us
                # per-trigger cost runs on two queues in parallel
                last = q >= NQUAD - 2
                act_cols = ACT_COPY_COLS if not last else D
                for pg in active_pages:
                    h_sb = hsb_pool.tile([128, D], bf16, tag="h_sb")
                    if act_cols > 0:
                        nc.scalar.copy(h_sb[:, 0:act_cols],
                                       hpages[pg][:, 0:act_cols])
                    if act_cols < D:
                        nc.vector.tensor_copy(h_sb[:, act_cols:D],
                                              hpages[pg][:, act_cols:D])
                    dma_eng = nc.sync if last else nc.gpsimd
                    for g, t0, tg in groups:
                        if g // 4 != pg:
                            continue
                        dma_eng.dma_start(
                            hout[t0:t0 + tg, :],
                            h_sb[32 * (g % 4):32 * (g % 4) + tg, :])

            # software pipeline: input DMAs lead by 2 quads (so their
            # triggers never queue behind compute-dependent work on the
            # Sync engine), and quad q-1's small-op chain and copies are
            # emitted INSIDE quad q's stat-pass stream (after groups 1 and
            # 5), so each engine's static order interleaves long-ready
            # chain work with streaming passes and outputs start early.
            LOOKAHEAD = 3
            loads = {}
            for q in range(min(LOOKAHEAD + 1, NQUAD)):
                loads[q] = emit_load(q)
            prev = None
            for q in range(NQUAD):
                if prev is None:
                    emit_passes(q, *loads[q])
                else:
                    pq = prev
                    box = {}

                    def part1(pq=pq, box=box):
                        box["pages"] = emit_chain(pq, *loads[pq])

                    def part2(pq=pq, box=box):
                        emit_copies(pq, loads[pq][0], *box["pages"])

                    emit_passes(q, *loads[q], part1=part1, part2=part2)
                if q + LOOKAHEAD + 1 < NQUAD:
                    loads[q + LOOKAHEAD + 1] = emit_load(q + LOOKAHEAD + 1)
                prev = q
            pages = emit_chain(prev, *loads[prev])
            emit_copies(prev, loads[prev][0], *pages)

    nc.compile()
    return nc


def _host_inputs(blocks, partial_block, proj_w, norm_w):
    """Slice + interleave per-core inputs (host-side, numpy only)."""
    import ml_dtypes
    bf = ml_dtypes.bfloat16
    blocks = np.ascontiguousarray(blocks, dtype=np.float32).reshape(N, B * T, D)
    partial = np.ascontiguousarray(partial_block, dtype=np.float32).reshape(B * T, D)
    w2 = (np.asarray(proj_w, np.float32) * np.asarray(norm_w, np.float32))
    w2b = np.ascontiguousarray(
        np.broadcast_to(w2.astype(bf), (ROWS, D)))
    oh = np.zeros((ROWS, GROUP), np.float32)
    for p in range(ROWS):
        oh[p, p % GROUP] = 1.0
    ohT = np.ascontiguousarray(oh.T)
    oh8 = np.ascontiguousarray(np.tile(oh, (1, QG)).astype(bf))

    pad_tok = NQUAD * QTOK
    in_maps = []
    for c in range(N_CORES):
        s = slice(c * TOK, (c + 1) * TOK)
        av = np.zeros((NB, pad_tok, D), bf)
        av[:N, :TOK] = blocks[:, s, :].astype(bf)
        av[N, :TOK] = partial[s, :].astype(bf)
        # vstack[q, 14n+t', g*D+d] = av[n, q*112 + g*14 + t', d]
        vst = av.reshape(NB, NQUAD, QG, GROUP, D)
        vst = np.ascontiguousarray(vst.transpose(1, 0, 3, 2, 4))
        vst = vst.reshape(NQUAD, ROWS, QG * D)
        in_maps.append({
            "vstack": vst,
            "w2b": w2b,
            "onehot": oh,
            "onehotT": ohT,
            "onehot8": oh8,
        })
    return in_maps


def kernel(blocks, partial_block, proj_w, norm_w):
    from concourse.bass_utils import run_bass_kernel_spmd

    if "nc" not in _CACHE:
        _CACHE["nc"] = build_nc()
    nc = _CACHE["nc"]
    in_maps = _host_inputs(blocks, partial_block, proj_w, norm_w)
    res = run_bass_kernel_spmd(nc, in_maps, core_ids=list(range(N_CORES)))
    h = np.concatenate(
        [np.asarray(res.results[c]["h"]).astype(np.float32)
         for c in range(N_CORES)], axis=0)
    return h.reshape(B, T, D)

